# revision 24
# baseline (speedup 1.0000x reference)
"""BlurOutwards Trainium2 kernel.

out = sum_{i=0}^{n-1} w_i * D^i(x),  w_i = 1/n (from -diff(linspace(1,0,n+1))),
D = 3x3 max-dilation with zero padding re-applied every step.

Sharding: pure data parallel. The 32*3 = 96 (batch, channel) images are split
into 8 slabs of 12; each NeuronCore processes its slab independently
(the dilation is spatially local, so no cross-core communication).

Per-core layout (two images per SBUF tile, fp16):
  img tile [128 partitions, 2 * 6*514]: partition p holds rows 4p..4p+3 of a
  512x512 image plus a halo row above/below; every row is 512 data columns
  followed by 2 zero guard columns, so +-1 column shifts read zeros at the
  image border (the reference's zero padding). The vertical pass uses free-dim
  shifts of +-514 (rows are stacked in the free dim); halo rows are refreshed
  once per dilation via two cross-partition SBUF->SBUF DMAs. Two images share
  every tile/op/DMA (halves instruction count and sem traffic).

Engines:
  DVE: 8 tensor_tensor max ops per image-pair per dilation (separable 3x3
       max; outer-pair-then-center order keeps all but the horizontal
       outer-pair op 4B-aligned => 2x mode). The first vertical op is split
       mid/top/bot so each edge piece waits on exactly one halo DMA (multi-
       sem waits cost an EVSEM sequence on the DVE sequencer), and m/D are
       split edge/mid so the next halo DMAs launch after D_edge and their
       ~3us flight hides behind m_mid/D_mid/t_mid.
  PE:  identity matmul accumulates each D^i into PSUM in fp32 (free: PE is
       otherwise idle and PSUM accumulation costs DVE nothing).
  ACT: PSUM->SBUF evacuation with the 1/n scale.
"""

import os
from contextlib import ExitStack

import numpy as np

N_CORES = 8
IMGS_TOTAL = 96          # 32 batch * 3 channels
IMGS_PER_CORE = IMGS_TOTAL // N_CORES  # 12
H = W = 512
R = 4                    # image rows per partition
P = 128                  # partitions (P*R == H)
ROWL = W + 2             # stored row length: 512 data + 2 zero guard cols
NROWS = R + 2            # stored rows per partition: halo + 4 data + halo
IMG_FREE = NROWS * ROWL  # 3084
DATA_LEN = R * ROWL      # 2056
PAIR = 2                 # images per tile (PSUM capacity caps the group at 2)

_CACHE: dict = {}


def _build(n: int):
    """Build the Bass module for n accumulation terms (n-1 dilations)."""
    import concourse.bacc as bacc
    import concourse.mybir as mybir
    import concourse.tile as tile
    from concourse.masks import make_identity

    dt = mybir.dt
    FP16 = dt.float16
    FP32 = dt.float32

    nc = bacc.Bacc("TRN2", debug=False, num_devices=N_CORES)

    x_t = nc.dram_tensor("x", [IMGS_PER_CORE, H, W], FP32, kind="ExternalInput")
    o_t = nc.dram_tensor("out", [IMGS_PER_CORE, H, W], FP32, kind="ExternalOutput")
    # [pair, p, i, (r c)]: partition p holds rows 4p..4p+3 of images (2q, 2q+1)
    x_ap = x_t.ap().rearrange("(q i) (p r) c -> q p i (r c)", i=PAIR, p=P)
    o_ap = o_t.ap().rearrange("(q i) (p r) c -> q p i (r c)", i=PAIR, p=P)

    scale = float(1.0 / n)

    def pairv(ap_2d, lo, hi):
        """[(pair-stride, 2), (1, hi-lo)] view of the same span in both images."""
        return ap_2d[:, :].rearrange("p (i f) -> p i f", i=PAIR)[:, :, lo:hi]

    def pair_rows(ap_2d, stride, off, nrows, cols=W):
        """[(stride*, 2), (rowl, nrows), (1, cols)] row-structured pair view."""
        return ap_2d[:, :].rearrange("p (i rc) -> p i rc", i=PAIR)[
            :, :, off : off + nrows * ROWL
        ].rearrange("p i (r c) -> p i r c", c=ROWL)[:, :, :, 0:cols]

    def erows(ap_2d, base, cols):
        """Edge rows {0, 3}: offsets {base, base + 3*ROWL} in a 3084-per-image
        tile (row stride 3*ROWL via the 2x1542 factoring)."""
        v = ap_2d[:, :].rearrange("p (i r c) -> p i r c", i=PAIR, c=3 * ROWL)
        return v[:, :, :, base : base + cols]

    def mrows(ap_2d, base, cols):
        """Mid rows {1, 2}: offsets {base, base + ROWL}."""
        v = ap_2d[:, :].rearrange("p (i f) -> p i f", i=PAIR)[
            :, :, base : base + 2 * ROWL
        ]
        return v.rearrange("p i (r c) -> p i r c", c=ROWL)[:, :, :, 0:cols]

    with tile.TileContext(nc) as tc, ExitStack() as ctx:
        ident_pool = ctx.enter_context(tc.tile_pool(name="ident", bufs=1))
        img_pool = ctx.enter_context(tc.tile_pool(name="img", bufs=3))
        t_pool = ctx.enter_context(tc.tile_pool(name="tpair", bufs=2))
        v_pool = ctx.enter_context(tc.tile_pool(name="vtmp", bufs=2))
        m_pool = ctx.enter_context(tc.tile_pool(name="mtmp", bufs=2))
        stage_pool = ctx.enter_context(tc.tile_pool(name="stage", bufs=2))
        outb_pool = ctx.enter_context(tc.tile_pool(name="outb", bufs=2))
        psum_pool = ctx.enter_context(tc.tile_pool(name="psum", bufs=1, space="PSUM"))

        identity = ident_pool.tile([P, P], FP16, name="identity")
        make_identity(nc, identity)

        for g in range(IMGS_PER_CORE // PAIR):
            # --- ingest: HBM fp32 -> staging -> fp16 image tile -------------
            stage = stage_pool.tile([P, PAIR * R * W], FP32, name="stage")
            nc.sync.dma_start(
                out=stage[:, :].rearrange("p (i f) -> p i f", i=PAIR), in_=x_ap[g]
            )

            # ping-pong image buffers: D^{it+1} is written to the other
            # buffer, so the PE accumulation of state `it` has a full
            # iteration of slack before its tile is overwritten (in-place
            # updates stall the DVE on the 8 serial matmuls every iteration)
            img0 = img_pool.tile([P, PAIR * IMG_FREE], FP16, name="img0")
            img1 = img_pool.tile([P, PAIR * IMG_FREE], FP16, name="img1")
            for img in (img0, img1):
                # zero guard cols; zero the image-boundary halo corners that
                # the halo DMAs never write (p0 top = rows "-1", p127
                # bottom). Only quadrant-aligned partition starts are legal,
                # so memset whole quadrants — the DMAs overwrite the
                # interior partitions anyway.
                nc.vector.memset(
                    img[:, :].rearrange("p (i r c) -> p i r c", i=PAIR, c=ROWL)[
                        :, :, :, W : W + 2
                    ],
                    0.0,
                )
                nc.vector.memset(pairv(img[0:32, :], 0, ROWL), 0.0)
                nc.vector.memset(
                    pairv(img[96:128, :], (NROWS - 1) * ROWL, IMG_FREE), 0.0
                )
            # cast data rows fp32 -> fp16 on the otherwise-idle ACT engine
            nc.scalar.activation(
                pair_rows(img0, IMG_FREE, ROWL, R),
                stage[:, :].rearrange("p (i r c) -> p i r c", i=PAIR, c=W),
                mybir.ActivationFunctionType.Copy,
            )

            psum_a = psum_pool.tile([P, R * W], FP32, name="acc_a")
            psum_b = psum_pool.tile([P, R * W], FP32, name="acc_b")
            tp = t_pool.tile([P, PAIR * DATA_LEN], FP16, name="tp")
            vt = v_pool.tile([P, PAIR * IMG_FREE], FP16, name="vt")
            nc.vector.memset(pairv(vt, 0, 2), 0.0)  # left pad for the H pass
            mt = m_pool.tile([P, PAIR * IMG_FREE], FP16, name="mt")

            VL = DATA_LEN + 2  # vt/mt per-image stride

            # --- n accumulations, n-1 dilations -----------------------------
            for it in range(n):
                cur = img0 if it % 2 == 0 else img1
                nxt = img1 if it % 2 == 0 else img0
                # acc += D^it(x): identity matmul into PSUM (fp32, free)
                for i, psum in ((0, psum_a), (1, psum_b)):
                    for k in range(R):
                        nc.tensor.matmul(
                            psum[:, k * W : (k + 1) * W],
                            identity[:, :],
                            cur[
                                :,
                                i * IMG_FREE
                                + (k + 1) * ROWL : i * IMG_FREE
                                + (k + 1) * ROWL
                                + W,
                            ],
                            start=(it == 0),
                            stop=(it == n - 1),
                        )
                if it == n - 1:
                    break

                # halo exchange for state `it` (edge rows of both images)
                nc.sync.dma_start(
                    out=pairv(cur[1:P, :], 0, ROWL),
                    in_=pairv(cur[0 : P - 1, :], R * ROWL, (R + 1) * ROWL),
                )
                nc.sync.dma_start(
                    out=pairv(cur[0 : P - 1, :], (NROWS - 1) * ROWL, IMG_FREE),
                    in_=pairv(cur[1:P, :], ROWL, 2 * ROWL),
                )

                # vertical 3-max, outer pair first: t_k = max(s_k, s_{k+2}).
                # mid (k=1,2) reads no halo rows -> it runs while the halo
                # DMAs fly; top (k=0) waits only the top DMA, bot (k=3) only
                # the bottom one.
                nc.vector.tensor_max(
                    pairv(tp, ROWL, 3 * ROWL),
                    pairv(cur, ROWL, 3 * ROWL),
                    pairv(cur, 3 * ROWL, 5 * ROWL),
                )
                nc.vector.tensor_max(
                    pairv(tp, 0, ROWL),
                    pairv(cur, 0, ROWL),
                    pairv(cur, 2 * ROWL, 3 * ROWL),
                )
                nc.vector.tensor_max(
                    pairv(tp, 3 * ROWL, DATA_LEN),
                    pairv(cur, 3 * ROWL, DATA_LEN),
                    pairv(cur, 5 * ROWL, IMG_FREE),
                )
                # v_k = max(t_k, s_{k+1})  (center row)
                nc.vector.tensor_max(
                    vt[:, :].rearrange("p (i f) -> p i f", i=PAIR)[
                        :, :, 2 : 2 + DATA_LEN
                    ],
                    pairv(tp, 0, DATA_LEN),
                    pairv(cur, ROWL, ROWL + DATA_LEN),
                )
                # horizontal 3-max: m[j] = max(v[j-1], v[j+1]) (odd offset,
                # the single unavoidable 1x op), then D = max(m, v).
                # Edge rows {0,3} go first: the next iteration's halo DMAs
                # need only D's edge rows, so they launch right after D_edge
                # and their flight hides behind m_mid/D_mid/t_mid.
                nc.vector.tensor_max(
                    erows(mt, 1, ROWL),
                    erows(vt, 0, ROWL),
                    erows(vt, 2, ROWL),
                )
                nc.vector.tensor_max(
                    erows(nxt, ROWL, W),
                    erows(mt, 2, W),
                    erows(vt, 2, W),
                )
                nc.vector.tensor_max(
                    pairv(mt, 1 + ROWL, 1 + 3 * ROWL),
                    pairv(vt, ROWL, 3 * ROWL),
                    pairv(vt, 2 + ROWL, 2 + 3 * ROWL),
                )
                nc.vector.tensor_max(
                    mrows(nxt, 2 * ROWL, W),
                    mrows(mt, 2 + ROWL, W),
                    mrows(vt, 2 + ROWL, W),
                )

            # --- evacuate: PSUM * (1/n) -> SBUF fp32 -> HBM -----------------
            outb = outb_pool.tile([P, PAIR * R * W], FP32, name="outb")
            for i, psum in ((0, psum_a), (1, psum_b)):
                nc.scalar.activation(
                    outb[:, i * R * W : (i + 1) * R * W],
                    psum[:, :],
                    mybir.ActivationFunctionType.Copy,
                    scale=scale,
                )
            nc.sync.dma_start(
                out=o_ap[g], in_=outb[:, :].rearrange("p (i f) -> p i f", i=PAIR)
            )

    nc.compile()
    return nc


def _get_nc(n: int):
    if n not in _CACHE:
        _CACHE[n] = _build(n)
    return _CACHE[n]


def kernel(in_tensor, n_pixels):
    n = int(n_pixels)
    x = np.ascontiguousarray(np.asarray(in_tensor, dtype=np.float32)).reshape(
        IMGS_TOTAL, H, W
    )
    if n <= 0:
        return np.zeros((32, 3, H, W), dtype=np.float32)
    if n == 1:
        # out = 1.0 * x exactly; skip the fp16 pipeline
        return x.reshape(32, 3, H, W).copy()

    from concourse.bass_utils import run_bass_kernel_spmd

    nc = _get_nc(n)
    in_maps = [
        {"x": np.ascontiguousarray(x[c * IMGS_PER_CORE : (c + 1) * IMGS_PER_CORE])}
        for c in range(N_CORES)
    ]
    trace = bool(int(os.environ.get("BLUR_TRACE", "0")))
    res = run_bass_kernel_spmd(
        nc, in_maps, core_ids=list(range(N_CORES)), trace=trace
    )
    if trace and res.exec_time_ns is not None:
        print(f"HW exec time: {res.exec_time_ns} ns")
        kernel.last_exec_time_ns = res.exec_time_ns
    out = np.concatenate([r["out"] for r in res.results], axis=0)
    return out.reshape(32, 3, H, W)


kernel.last_exec_time_ns = None


# revision 30
# speedup vs baseline: 1.0063x; 1.0063x over previous
"""BlurOutwards Trainium2 kernel.

out = sum_{i=0}^{n-1} w_i * D^i(x),  w_i = 1/n (from -diff(linspace(1,0,n+1))),
D = 3x3 max-dilation with zero padding re-applied every step.

Sharding: pure data parallel. The 32*3 = 96 (batch, channel) images are split
into 8 slabs of 12; each NeuronCore processes its slab independently
(the dilation is spatially local, so no cross-core communication).

Per-core layout (two images per SBUF tile, fp16):
  img tile [128 partitions, 2 * 6*514]: partition p holds rows 4p..4p+3 of a
  512x512 image plus a halo row above/below; every row is 512 data columns
  followed by 2 zero guard columns, so +-1 column shifts read zeros at the
  image border (the reference's zero padding). The vertical pass uses free-dim
  shifts of +-514 (rows are stacked in the free dim); halo rows are refreshed
  once per dilation via two cross-partition SBUF->SBUF DMAs. Two images share
  every tile/op/DMA (halves instruction count and sem traffic).

Engines:
  DVE: 8 tensor_tensor max ops per image-pair per dilation (separable 3x3
       max; outer-pair-then-center order keeps all but the horizontal
       outer-pair op 4B-aligned => 2x mode). The first vertical op is split
       mid/top/bot so each edge piece waits on exactly one halo DMA (multi-
       sem waits cost an EVSEM sequence on the DVE sequencer), and m/D are
       split edge/mid so the next halo DMAs launch after D_edge and their
       ~3us flight hides behind m_mid/D_mid/t_mid.
  PE:  identity matmul accumulates each D^i into PSUM in fp32 (free: PE is
       otherwise idle and PSUM accumulation costs DVE nothing).
  ACT: PSUM->SBUF evacuation with the 1/n scale.
"""

import os
from contextlib import ExitStack

import numpy as np

N_CORES = 8
IMGS_TOTAL = 96          # 32 batch * 3 channels
IMGS_PER_CORE = IMGS_TOTAL // N_CORES  # 12
H = W = 512
R = 4                    # image rows per partition
P = 128                  # partitions (P*R == H)
ROWL = W + 2             # stored row length: 512 data + 2 zero guard cols
NROWS = R + 2            # stored rows per partition: halo + 4 data + halo
IMG_FREE = NROWS * ROWL  # 3084
DATA_LEN = R * ROWL      # 2056
PAIR = 2                 # images per tile (PSUM capacity caps the group at 2)

_CACHE: dict = {}


def _build(n: int):
    """Build the Bass module for n accumulation terms (n-1 dilations)."""
    import concourse.bacc as bacc
    import concourse.mybir as mybir
    import concourse.tile as tile
    from concourse.masks import make_identity

    dt = mybir.dt
    FP16 = dt.float16
    FP32 = dt.float32

    nc = bacc.Bacc("TRN2", debug=False, num_devices=N_CORES)

    x_t = nc.dram_tensor("x", [IMGS_PER_CORE, H, W], FP32, kind="ExternalInput")
    o_t = nc.dram_tensor("out", [IMGS_PER_CORE, H, W], FP32, kind="ExternalOutput")
    # [pair, p, i, (r c)]: partition p holds rows 4p..4p+3 of images (2q, 2q+1)
    x_ap = x_t.ap().rearrange("(q i) (p r) c -> q p i (r c)", i=PAIR, p=P)
    o_ap = o_t.ap().rearrange("(q i) (p r) c -> q p i (r c)", i=PAIR, p=P)

    scale = float(1.0 / n)

    def pairv(ap_2d, lo, hi):
        """[(pair-stride, 2), (1, hi-lo)] view of the same span in both images."""
        return ap_2d[:, :].rearrange("p (i f) -> p i f", i=PAIR)[:, :, lo:hi]

    def pair_rows(ap_2d, stride, off, nrows, cols=W):
        """[(stride*, 2), (rowl, nrows), (1, cols)] row-structured pair view."""
        return ap_2d[:, :].rearrange("p (i rc) -> p i rc", i=PAIR)[
            :, :, off : off + nrows * ROWL
        ].rearrange("p i (r c) -> p i r c", c=ROWL)[:, :, :, 0:cols]

    def erows(ap_2d, base, cols):
        """Edge rows {0, 3}: offsets {base, base + 3*ROWL} in a 3084-per-image
        tile (row stride 3*ROWL via the 2x1542 factoring)."""
        v = ap_2d[:, :].rearrange("p (i r c) -> p i r c", i=PAIR, c=3 * ROWL)
        return v[:, :, :, base : base + cols]

    def mrows(ap_2d, base, cols):
        """Mid rows {1, 2}: offsets {base, base + ROWL}."""
        v = ap_2d[:, :].rearrange("p (i f) -> p i f", i=PAIR)[
            :, :, base : base + 2 * ROWL
        ]
        return v.rearrange("p i (r c) -> p i r c", c=ROWL)[:, :, :, 0:cols]

    with tile.TileContext(nc) as tc, ExitStack() as ctx:
        ident_pool = ctx.enter_context(tc.tile_pool(name="ident", bufs=1))
        img_pool = ctx.enter_context(tc.tile_pool(name="img", bufs=3))
        t_pool = ctx.enter_context(tc.tile_pool(name="tpair", bufs=2))
        v_pool = ctx.enter_context(tc.tile_pool(name="vtmp", bufs=2))
        m_pool = ctx.enter_context(tc.tile_pool(name="mtmp", bufs=2))
        stage_pool = ctx.enter_context(tc.tile_pool(name="stage", bufs=2))
        outb_pool = ctx.enter_context(tc.tile_pool(name="outb", bufs=2))
        psum_pool = ctx.enter_context(tc.tile_pool(name="psum", bufs=1, space="PSUM"))

        identity = ident_pool.tile([P, P], FP16, name="identity")
        make_identity(nc, identity)

        for g in range(IMGS_PER_CORE // PAIR):
            # --- ingest: HBM fp32 -> staging -> fp16 image tile -------------
            stage = stage_pool.tile([P, PAIR * R * W], FP32, name="stage")
            nc.sync.dma_start(
                out=stage[:, :].rearrange("p (i f) -> p i f", i=PAIR), in_=x_ap[g]
            )

            # ping-pong image buffers: D^{it+1} is written to the other
            # buffer, so the PE accumulation of state `it` has a full
            # iteration of slack before its tile is overwritten (in-place
            # updates stall the DVE on the 8 serial matmuls every iteration)
            img0 = img_pool.tile([P, PAIR * IMG_FREE], FP16, name="img0")
            img1 = img_pool.tile([P, PAIR * IMG_FREE], FP16, name="img1")
            for img in (img0, img1):
                # zero guard cols; zero the image-boundary halo corners that
                # the halo DMAs never write (p0 top = rows "-1", p127
                # bottom). Only quadrant-aligned partition starts are legal,
                # so memset whole quadrants — the DMAs overwrite the
                # interior partitions anyway.
                nc.gpsimd.memset(
                    img[:, :].rearrange("p (i r c) -> p i r c", i=PAIR, c=ROWL)[
                        :, :, :, W : W + 2
                    ],
                    0.0,
                )
                nc.gpsimd.memset(pairv(img[0:32, :], 0, ROWL), 0.0)
                nc.gpsimd.memset(
                    pairv(img[96:128, :], (NROWS - 1) * ROWL, IMG_FREE), 0.0
                )
            # cast data rows fp32 -> fp16 on the otherwise-idle ACT engine
            nc.scalar.activation(
                pair_rows(img0, IMG_FREE, ROWL, R),
                stage[:, :].rearrange("p (i r c) -> p i r c", i=PAIR, c=W),
                mybir.ActivationFunctionType.Copy,
            )

            psum_a = psum_pool.tile([P, R * W], FP32, name="acc_a")
            psum_b = psum_pool.tile([P, R * W], FP32, name="acc_b")
            tp = t_pool.tile([P, PAIR * DATA_LEN], FP16, name="tp")
            vt = v_pool.tile([P, PAIR * IMG_FREE], FP16, name="vt")
            nc.gpsimd.memset(pairv(vt, 0, 2), 0.0)  # left pad for the H pass
            mt = m_pool.tile([P, PAIR * IMG_FREE], FP16, name="mt")

            VL = DATA_LEN + 2  # vt/mt per-image stride

            # --- n accumulations, n-1 dilations -----------------------------
            for it in range(n):
                cur = img0 if it % 2 == 0 else img1
                nxt = img1 if it % 2 == 0 else img0
                # acc += D^it(x): identity matmul into PSUM (fp32, free)
                for i, psum in ((0, psum_a), (1, psum_b)):
                    for k in range(R):
                        nc.tensor.matmul(
                            psum[:, k * W : (k + 1) * W],
                            identity[:, :],
                            cur[
                                :,
                                i * IMG_FREE
                                + (k + 1) * ROWL : i * IMG_FREE
                                + (k + 1) * ROWL
                                + W,
                            ],
                            start=(it == 0),
                            stop=(it == n - 1),
                        )
                if it == n - 1:
                    break

                # halo exchange for state `it` (edge rows of both images)
                nc.sync.dma_start(
                    out=pairv(cur[1:P, :], 0, ROWL),
                    in_=pairv(cur[0 : P - 1, :], R * ROWL, (R + 1) * ROWL),
                )
                nc.sync.dma_start(
                    out=pairv(cur[0 : P - 1, :], (NROWS - 1) * ROWL, IMG_FREE),
                    in_=pairv(cur[1:P, :], ROWL, 2 * ROWL),
                )

                # vertical 3-max, outer pair first: t_k = max(s_k, s_{k+2}).
                # mid (k=1,2) reads no halo rows -> it runs while the halo
                # DMAs fly; top (k=0) waits only the top DMA, bot (k=3) only
                # the bottom one.
                nc.vector.tensor_max(
                    pairv(tp, ROWL, 3 * ROWL),
                    pairv(cur, ROWL, 3 * ROWL),
                    pairv(cur, 3 * ROWL, 5 * ROWL),
                )
                nc.vector.tensor_max(
                    pairv(tp, 0, ROWL),
                    pairv(cur, 0, ROWL),
                    pairv(cur, 2 * ROWL, 3 * ROWL),
                )
                nc.vector.tensor_max(
                    pairv(tp, 3 * ROWL, DATA_LEN),
                    pairv(cur, 3 * ROWL, DATA_LEN),
                    pairv(cur, 5 * ROWL, IMG_FREE),
                )
                # v_k = max(t_k, s_{k+1})  (center row)
                nc.vector.tensor_max(
                    vt[:, :].rearrange("p (i f) -> p i f", i=PAIR)[
                        :, :, 2 : 2 + DATA_LEN
                    ],
                    pairv(tp, 0, DATA_LEN),
                    pairv(cur, ROWL, ROWL + DATA_LEN),
                )
                # horizontal 3-max: m[j] = max(v[j-1], v[j+1]) (odd offset,
                # the single unavoidable 1x op), then D = max(m, v).
                # Edge rows {0,3} go first: the next iteration's halo DMAs
                # need only D's edge rows, so they launch right after D_edge
                # and their flight hides behind m_mid/D_mid/t_mid.
                nc.vector.tensor_max(
                    erows(mt, 1, ROWL),
                    erows(vt, 0, ROWL),
                    erows(vt, 2, ROWL),
                )
                nc.vector.tensor_max(
                    erows(nxt, ROWL, W),
                    erows(mt, 2, W),
                    erows(vt, 2, W),
                )
                nc.vector.tensor_max(
                    pairv(mt, 1 + ROWL, 1 + 3 * ROWL),
                    pairv(vt, ROWL, 3 * ROWL),
                    pairv(vt, 2 + ROWL, 2 + 3 * ROWL),
                )
                nc.vector.tensor_max(
                    mrows(nxt, 2 * ROWL, W),
                    mrows(mt, 2 + ROWL, W),
                    mrows(vt, 2 + ROWL, W),
                )

            # --- evacuate: PSUM * (1/n) -> SBUF fp32 -> HBM -----------------
            outb = outb_pool.tile([P, PAIR * R * W], FP32, name="outb")
            for i, psum in ((0, psum_a), (1, psum_b)):
                nc.scalar.activation(
                    outb[:, i * R * W : (i + 1) * R * W],
                    psum[:, :],
                    mybir.ActivationFunctionType.Copy,
                    scale=scale,
                )
            nc.sync.dma_start(
                out=o_ap[g], in_=outb[:, :].rearrange("p (i f) -> p i f", i=PAIR)
            )

    nc.compile()
    return nc


def _get_nc(n: int):
    if n not in _CACHE:
        _CACHE[n] = _build(n)
    return _CACHE[n]


def kernel(in_tensor, n_pixels):
    n = int(n_pixels)
    x = np.ascontiguousarray(np.asarray(in_tensor, dtype=np.float32)).reshape(
        IMGS_TOTAL, H, W
    )
    if n <= 0:
        return np.zeros((32, 3, H, W), dtype=np.float32)
    if n == 1:
        # out = 1.0 * x exactly; skip the fp16 pipeline
        return x.reshape(32, 3, H, W).copy()

    from concourse.bass_utils import run_bass_kernel_spmd

    nc = _get_nc(n)
    in_maps = [
        {"x": np.ascontiguousarray(x[c * IMGS_PER_CORE : (c + 1) * IMGS_PER_CORE])}
        for c in range(N_CORES)
    ]
    trace = bool(int(os.environ.get("BLUR_TRACE", "0")))
    res = run_bass_kernel_spmd(
        nc, in_maps, core_ids=list(range(N_CORES)), trace=trace
    )
    if trace and res.exec_time_ns is not None:
        print(f"HW exec time: {res.exec_time_ns} ns")
        kernel.last_exec_time_ns = res.exec_time_ns
    out = np.concatenate([r["out"] for r in res.results], axis=0)
    return out.reshape(32, 3, H, W)


kernel.last_exec_time_ns = None
